# revision 1
# baseline (speedup 1.0000x reference)
"""Trainium2 Bass kernel for nn_CrossDomainFusion.

Data-parallel over batch: core b handles batch element b (B=8, 8 cores).

Math (per batch):
  time branch: ConvTranspose1d(stride 2, pad 1, K=4) then Linear(256->512).
    Folded into two strided projections with fused weights:
      H_time[2t]   = x[t] @ (W1@time_w) + x[t-1] @ (W3@time_w) + bias_h
      H_time[2t+1] = x[t+1] @ (W0@time_w) + x[t] @ (W2@time_w) + bias_h
  spec branch: H_spec = spec.reshape(192,2048).T @ spec_w + spec_b
  S[t,s] = <H_time[t], H_spec[s]> / sqrt(512);  E = exp(S)
  out[t, :512]  = (E @ H_spec)[t]   / sum_s E[t,s]
  out[s, 512:]  = (E.T @ H_time)[s] / sum_t E[t,s]

Device pipeline per core (t' denotes [even | odd] block-permuted time order):
  1) Ht_T [h,t'] and Hs_T [h,s] via fp32r (TF32) matmuls from native layouts
  2) Ht [t',h], Hs [s,h] in bf16 (attention values)
  3) S_st tiles = Hs_T^T @ Ht_T (fp32r), exp on ScalarE (accum_out -> D_spec)
  4) E_ts tiles via PE transpose of E_st (accum_out on copies -> D_time)
  5) fused_time = (E_st as lhsT) @ Hs_bf ; fused_spec = (E_ts as lhsT) @ Ht_bf
     normalized by reciprocal row sums during PSUM->SBUF copy, DMA to output
"""

import numpy as np

import concourse.bass as bass
import concourse.tile as tile
from concourse import bacc, mybir
from concourse.bass_utils import run_bass_kernel_spmd
from concourse.masks import make_identity

F32 = mybir.dt.float32
F32R = mybir.dt.float32r
BF16 = mybir.dt.bfloat16

B, T, TD, SD, HD = 8, 1024, 256, 192, 512
T2 = 2 * T            # 2048
NT = T2 // 128        # 16 tiles of 128 along t'/s
SCALE = float(1.0 / np.sqrt(np.float32(HD)))

LAST_RESULT = None    # BassKernelResults of the most recent run (for test.py)


def _tf32_round(x: np.ndarray) -> np.ndarray:
    """Round fp32 to TF32 (10-bit mantissa, round-to-nearest-even)."""
    u = np.ascontiguousarray(x, dtype=np.float32).view(np.uint32)
    r = (u + np.uint32(0xFFF) + ((u >> np.uint32(13)) & np.uint32(1))) & np.uint32(
        0xFFFFE000
    )
    return r.view(np.float32)


def _emit(nc, aps, repeats=1):
    with tile.TileContext(nc) as tc:
        for _ in range(repeats):
            _emit_body(nc, tc, aps)


def _emit_body(nc, tc, aps):
    xt_d = aps["xt"]
    spr_d = aps["specr"]
    out_d = aps["out"]

    if True:
        with tc.tile_pool(name="const", bufs=1) as pconst, \
             tc.tile_pool(name="persist", bufs=1) as pp, \
             tc.tile_pool(name="stage", bufs=3) as stg, \
             tc.tile_pool(name="pmm", bufs=4, space="PSUM") as pmm, \
             tc.tile_pool(name="ptp", bufs=4, space="PSUM") as ptp:

            ident = pconst.tile([128, 128], BF16, tag="ident")
            make_identity(nc, ident[:])
            ident_f = pconst.tile([128, 128], F32, tag="ident_f")
            make_identity(nc, ident_f[:])
            identr = pconst.tile([128, 128], F32R, tag="identr")
            nc.vector.tensor_copy(identr[:], ident_f[:])

            HtBF = pp.tile([128, NT, HD], BF16, tag="htbf")
            HsBF = pp.tile([128, NT, HD], BF16, tag="hsbf")
            DSP = pp.tile([128, NT, 4], F32, tag="dsp")
            DTP = pp.tile([128, NT, NT // 4], F32, tag="dtp")
            DS = pp.tile([128, NT], F32, tag="ds")
            DT = pp.tile([128, NT], F32, tag="dt")
            RDS = pp.tile([128, NT], F32, tag="rds")
            RDT = pp.tile([128, NT], F32, tag="rdt")

            with tc.tile_pool(name="hT", bufs=1) as phT:
                HtT = phT.tile([128, 4, T2], F32R, tag="htT")
                HsT = phT.tile([128, 4, T2], F32R, tag="hsT")

                with tc.tile_pool(name="pin", bufs=1) as pin:
                    # ---- loads ----
                    XT = pin.tile([128, 2, T], F32R, tag="xt")
                    XTm1 = pin.tile([128, 2, T], F32R, tag="xtm1")
                    XTp1 = pin.tile([128, 2, T], F32R, tag="xtp1")
                    SPR = pin.tile([128, 2, T2], F32R, tag="spr")
                    WS = {}
                    for nm in ("wae", "wbe", "wao", "wbo", "wsp"):
                        WS[nm] = pin.tile([128, 2, HD], F32R, tag=nm, name=nm)
                    BH = pin.tile([128, 4], F32, tag="bh")
                    BS = pin.tile([128, 4], F32, tag="bs")

                    for hc in range(4):
                        nc.sync.dma_start(out=BH[:, hc:hc + 1], in_=aps["bh"][hc, :])
                        nc.sync.dma_start(out=BS[:, hc:hc + 1], in_=aps["bs"][hc, :])
                    for ci in range(2):
                        rows = slice(128 * ci, 128 * ci + 128)
                        for nm in ("wae", "wbe"):
                            nc.sync.dma_start(out=WS[nm][:, ci, :],
                                              in_=aps[nm][rows, :])
                    for csl in (slice(0, 512), slice(512, 1024)):
                        for ci in range(2):
                            rows = slice(128 * ci, 128 * ci + 128)
                            nc.sync.dma_start(out=XT[:, ci, csl],
                                              in_=xt_d[rows, csl])
                            nc.sync.dma_start(out=XTm1[:, ci, csl],
                                              in_=aps["xtm1"][rows, csl])
                            nc.sync.dma_start(out=XTp1[:, ci, csl],
                                              in_=aps["xtp1"][rows, csl])
                    for ci in range(2):
                        rows = slice(128 * ci, 128 * ci + 128)
                        for nm in ("wao", "wbo"):
                            nc.sync.dma_start(out=WS[nm][:, ci, :],
                                              in_=aps[nm][rows, :])
                    nc.sync.dma_start(out=WS["wsp"][:, 0, :], in_=aps["wsp"][0:128, :])
                    nc.sync.dma_start(out=WS["wsp"][0:64, 1, :], in_=aps["wsp"][128:192, :])
                    nc.sync.dma_start(out=SPR[:, 0, :], in_=spr_d[0:128, :])
                    nc.sync.dma_start(out=SPR[0:64, 1, :], in_=spr_d[128:192, :])

                    # ---- phase 1: Ht_T [h, t'] fp32r ----
                    # even half cols 0..1023 (t'=t_in), odd half cols 1024..2047
                    for hc in range(4):
                        hsl = slice(128 * hc, 128 * hc + 128)
                        for half, terms in enumerate(
                            (((WS["wae"], XT), (WS["wbe"], XTm1)),
                             ((WS["wao"], XTp1), (WS["wbo"], XT)))):
                            for tc2 in range(2):
                                tsl = slice(512 * tc2, 512 * tc2 + 512)
                                ps = pmm.tile([128, 512], F32, tag="ps")
                                mm = []
                                for ci in range(2):
                                    for (w, x) in terms:
                                        mm.append((w[:, ci, hsl], x[:, ci, tsl]))
                                for q, (lh, rh) in enumerate(mm):
                                    nc.tensor.matmul(ps[:], lh, rh,
                                                     start=(q == 0), stop=(q == 3))
                                dst = HtT[:, hc, 1024 * half + 512 * tc2:
                                          1024 * half + 512 * tc2 + 512]
                                nc.scalar.activation(
                                    dst, ps[:],
                                    mybir.ActivationFunctionType.Identity,
                                    bias=BH[:, hc:hc + 1])

                    # ---- phase 2: Hs_T [h, s] fp32r ----
                    for hc in range(4):
                        hsl = slice(128 * hc, 128 * hc + 128)
                        for sc in range(4):
                            ssl = slice(512 * sc, 512 * sc + 512)
                            ps = pmm.tile([128, 512], F32, tag="ps")
                            for ci, kk in enumerate((128, 64)):
                                nc.tensor.matmul(ps[:], WS["wsp"][0:kk, ci, hsl],
                                                 SPR[0:kk, ci, ssl],
                                                 start=(ci == 0), stop=(ci == 1))
                            nc.scalar.activation(
                                HsT[:, hc, ssl], ps[:],
                                mybir.ActivationFunctionType.Identity,
                                bias=BS[:, hc:hc + 1])

                # pin closed: input tiles freed

                # ---- phases 3/4: value-side H in bf16 by PE-transposing the
                # already-biased Ht_T/Hs_T (4 transposes batched per PSUM
                # bank -> one wide copy each) ----
                for (src, dstbf) in ((HtT, HtBF), (HsT, HsBF)):
                    for j in range(NT):
                        ps = ptp.tile([128, 512], F32R, tag="tp", name="psr")
                        for hc in range(4):
                            nc.tensor.transpose(
                                ps[:, 128 * hc:128 * hc + 128],
                                src[:, hc, 128 * j:128 * j + 128], identr[:])
                        if j % 4 == 0:
                            nc.scalar.activation(
                                dstbf[:, j, :], ps[:].bitcast(F32),
                                mybir.ActivationFunctionType.Identity)
                        else:
                            nc.vector.tensor_copy(dstbf[:, j, :],
                                                  ps[:].bitcast(F32))

                with tc.tile_pool(name="pest", bufs=1) as pest:
                    EST = pest.tile([128, NT, T2], BF16, tag="est")

                    # ---- phase 5: scores + exp -> E_st [s, t'], D_spec ----
                    for i in range(NT):
                        ssl = slice(128 * i, 128 * i + 128)
                        for tc4 in range(4):
                            tsl = slice(512 * tc4, 512 * tc4 + 512)
                            ps = pmm.tile([128, 512], F32, tag="ps")
                            for hc in range(4):
                                nc.tensor.matmul(ps[:], HsT[:, hc, ssl],
                                                 HtT[:, hc, tsl],
                                                 start=(hc == 0), stop=(hc == 3))
                            nc.scalar.activation(
                                EST[:, i, tsl], ps[:],
                                mybir.ActivationFunctionType.Exp,
                                scale=SCALE,
                                accum_out=DSP[:, i, tc4:tc4 + 1])
                    nc.vector.tensor_reduce(DS[:], DSP[:],
                                            mybir.AxisListType.X,
                                            mybir.AluOpType.add)
                    nc.vector.reciprocal(RDS[:], DS[:])

                    # ---- phase 6: fused_spec = E_ts.T @ Ht with inline PE
                    # transposes of E_st tiles. Four transposes (same t-chunk
                    # j, 4 adjacent s-tiles) batch into one PSUM bank -> one
                    # wide copy whose accum_out is still a valid D_time
                    # partial (all quadrants share t partitions). Each wide
                    # ets tile then feeds 4 s-blocks' matmuls. ----
                    for g in range(NT // 4):
                        etss = []
                        for j in range(NT):
                            tp = ptp.tile([128, 512], BF16, tag="tp", name="tp6")
                            for r in range(4):
                                i = 4 * g + r
                                nc.tensor.transpose(
                                    tp[:, 128 * r:128 * r + 128],
                                    EST[:, i, 128 * j:128 * j + 128], ident[:])
                            ets = stg.tile([128, 512], BF16, tag="ets", bufs=20,
                                           name="ets")
                            if j % 4 == 0:
                                nc.scalar.activation(
                                    ets[:], tp[:],
                                    mybir.ActivationFunctionType.Identity,
                                    accum_out=DTP[:, j, g:g + 1])
                            else:
                                nc.vector.tensor_scalar(
                                    ets[:], tp[:], 1.0, None,
                                    mybir.AluOpType.mult,
                                    mybir.AluOpType.add,
                                    accum_out=DTP[:, j, g:g + 1])
                            etss.append(ets)
                        for r in range(4):
                            i = 4 * g + r
                            ps = pmm.tile([128, 512], F32, tag="ps")
                            for j in range(NT):
                                nc.tensor.matmul(
                                    ps[:], etss[j][:, 128 * r:128 * r + 128],
                                    HtBF[:, j, :],
                                    start=(j == 0), stop=(j == NT - 1))
                            st = stg.tile([128, 512], F32, tag="stage")
                            nc.vector.tensor_scalar_mul(st[:], ps[:],
                                                        RDS[:, i:i + 1])
                            nc.sync.dma_start(
                                out=out_d[128 * i:128 * i + 128, 512:1024],
                                in_=st[:])
                    nc.vector.tensor_reduce(DT[:], DTP[:],
                                            mybir.AxisListType.X,
                                            mybir.AluOpType.add)
                    nc.vector.reciprocal(RDT[:], DT[:])

                    # ---- phase 7: fused_time = E_st.T @ Hs, normalize ----
                    for j in range(NT):
                        ps = pmm.tile([128, 512], F32, tag="ps")
                        for i in range(NT):
                            nc.tensor.matmul(ps[:], EST[:, i, 128 * j:128 * j + 128],
                                             HsBF[:, i, :],
                                             start=(i == 0), stop=(i == NT - 1))
                        st = stg.tile([128, 512], F32, tag="stage")
                        nc.vector.tensor_scalar_mul(st[:], ps[:], RDT[:, j:j + 1])
                        start = 256 * j if j < 8 else 256 * (j - 8) + 1
                        dst = out_d[start:start + 255:2, 0:512]
                        nc.sync.dma_start(out=dst, in_=st[:])


_PROGRAM = None


def _build_program(repeats=1):
    global _PROGRAM
    if _PROGRAM is not None and repeats == 1:
        return _PROGRAM
    nc = bacc.Bacc("TRN2", target_bir_lowering=False, debug=False, num_devices=8)
    aps = {
        "xt": nc.dram_tensor("xt", [TD, T], F32R, kind="ExternalInput").ap(),
        "xtm1": nc.dram_tensor("xtm1", [TD, T], F32R, kind="ExternalInput").ap(),
        "xtp1": nc.dram_tensor("xtp1", [TD, T], F32R, kind="ExternalInput").ap(),
        "specr": nc.dram_tensor("specr", [SD, T2], F32R, kind="ExternalInput").ap(),
        "wae": nc.dram_tensor("wae", [TD, HD], F32R, kind="ExternalInput").ap(),
        "wbe": nc.dram_tensor("wbe", [TD, HD], F32R, kind="ExternalInput").ap(),
        "wao": nc.dram_tensor("wao", [TD, HD], F32R, kind="ExternalInput").ap(),
        "wbo": nc.dram_tensor("wbo", [TD, HD], F32R, kind="ExternalInput").ap(),
        "wsp": nc.dram_tensor("wsp", [SD, HD], F32R, kind="ExternalInput").ap(),
        "bh": nc.dram_tensor("bh", [4, 128], F32, kind="ExternalInput").ap(),
        "bs": nc.dram_tensor("bs", [4, 128], F32, kind="ExternalInput").ap(),
        "bhr": nc.dram_tensor("bhr", [1, HD], F32R, kind="ExternalInput").ap(),
        "bsr": nc.dram_tensor("bsr", [1, HD], F32R, kind="ExternalInput").ap(),
        "out": nc.dram_tensor("out", [T2, 2 * HD], F32, kind="ExternalOutput").ap(),
    }
    _emit(nc, aps, repeats=repeats)
    nc.compile()
    if repeats == 1:
        _PROGRAM = nc
    return nc


def _prep_in_maps(time_features, spec_features, conv_w, conv_b, time_w, time_b,
                  spec_w, spec_b):
    time_features = np.asarray(time_features, dtype=np.float32)
    spec_features = np.asarray(spec_features, dtype=np.float32)
    conv_w = np.asarray(conv_w, dtype=np.float32)
    conv_b = np.asarray(conv_b, dtype=np.float32)
    time_w = np.asarray(time_w, dtype=np.float32)
    time_b = np.asarray(time_b, dtype=np.float32)
    spec_w = np.asarray(spec_w, dtype=np.float32)
    spec_b = np.asarray(spec_b, dtype=np.float32)

    # fold conv-transpose into per-parity projection weights (exact algebra)
    Wk = [conv_w[:, :, k] for k in range(4)]
    wae = _tf32_round(Wk[1] @ time_w)
    wbe = _tf32_round(Wk[3] @ time_w)
    wao = _tf32_round(Wk[0] @ time_w)
    wbo = _tf32_round(Wk[2] @ time_w)
    bias_h = (conv_b @ time_w + time_b).astype(np.float32)
    wsp = _tf32_round(spec_w)

    shared = {
        "wae": wae, "wbe": wbe, "wao": wao, "wbo": wbo, "wsp": wsp,
        "bh": np.ascontiguousarray(bias_h.reshape(4, 128)),
        "bs": np.ascontiguousarray(spec_b.reshape(4, 128)),
        "bhr": _tf32_round(bias_h.reshape(1, HD)),
        "bsr": _tf32_round(spec_b.reshape(1, HD)),
    }
    in_maps = []
    for b in range(B):
        m = dict(shared)
        xt = _tf32_round(np.ascontiguousarray(time_features[b].T))
        m["xt"] = xt
        xtm1 = np.zeros_like(xt)
        xtm1[:, 1:] = xt[:, :-1]
        m["xtm1"] = xtm1
        xtp1 = np.zeros_like(xt)
        xtp1[:, :-1] = xt[:, 1:]
        m["xtp1"] = xtp1
        m["specr"] = _tf32_round(
            np.ascontiguousarray(spec_features[b].reshape(SD, T2)))
        in_maps.append(m)
    return in_maps


def kernel(**inputs):
    in_maps = _prep_in_maps(**inputs)
    nc = _build_program()
    res = run_bass_kernel_spmd(nc, in_maps, list(range(B)))
    global LAST_RESULT
    LAST_RESULT = res
    return np.stack([res.results[b]["out"] for b in range(B)], axis=0)

